# revision 5
# baseline (speedup 1.0000x reference)
"""CBAM3D Trainium2 kernel (8 NeuronCores, SPMD).

Reference computation (per batch sample b):
  avg_pool[c] = mean_{d,h,w} x ; max_pool[c] = max_{d,h,w} x
  ca = sigmoid(relu(avg@w1+b1)@w2+b2) + sigmoid(relu(max@w1+b1)@w2+b2)
  refined = x * ca[c]
  P = [mean_c refined, max_c refined]            # [D,H,W,2]
  sa = sigmoid(conv3d_same(P, conv_w))           # 7x7x7x2 -> 1
  out = refined * sa

Sharding: core i handles sample b=i//2, D-half half=i%2 (32 planes + 3-plane
halos, host-padded into a uniform [38,H,W,C] slab). The only cross-core
traffic is a pair-wise AllReduce of the per-channel sum/max stats (256B).

Per-core pipeline:
  pass1: stream x (f32), convert to a bf16 SBUF cache, accumulate channel
         sum (PE matmul vs ones) + channel max (DVE running max)
  AllReduce(add), AllReduce(max) over {2i,2i+1}; tiny MLP on device -> ca
  phase2a: refined = cache*ca in-place; per-plane channel sum/max -> pooled
         map [(ci,h'), slot, w+pad] via permutation matmuls (halo planes are
         read from the f32 slab directly)
  conv:  49 accumulating matmuls per 8-plane block with host-prebuilt
         "band matrices" (kh,ci folded into K=128) -> sigmoid -> sa
  apply: out = refined * sa, DMA out f32
"""

from dataclasses import dataclass

import numpy as np
import ml_dtypes

import concourse.bass as bass
import concourse.tile as tile
import concourse.mybir as mybir
from concourse import bacc, bass_isa

F32 = mybir.dt.float32
F32R = mybir.dt.float32r
BF16 = mybir.dt.bfloat16
AX = mybir.AxisListType
OP = mybir.AluOpType
ACT = mybir.ActivationFunctionType


@dataclass(frozen=True)
class Cfg:
    H: int = 64
    W: int = 64
    C: int = 64
    D_LOC: int = 32          # own planes per core
    HID: int = 4             # C // reduction_ratio
    KS: int = 7
    N_CORES: int = 8
    use_collectives: bool = True
    stop_after: str = "full"   # pass1 | mlp | pool | conv | full

    @property
    def HALO(self):
        return self.KS // 2

    @property
    def S(self):
        return self.D_LOC + 2 * self.HALO   # slots in the slab / pooled map

    @property
    def P(self):
        return 2 * self.H                    # partition dim of pair tiles

    @property
    def WP(self):
        return self.W + 2 * self.HALO        # padded pooled-map width

    @property
    def D_TOT(self):
        return 2 * self.D_LOC                # full-sample depth (2 shards)


FULL = Cfg()


def _bc(ap, shape, axis):
    """broadcast ap (by unsqueezing `axis`) to `shape`"""
    return ap.unsqueeze(axis).broadcast_to(shape)


def build_cbam(nc, cfg: Cfg):
    H, W, C = cfg.H, cfg.W, cfg.C
    P, S, WP, HALO = cfg.P, cfg.S, cfg.WP, cfg.HALO
    D_LOC, HID, KS = cfg.D_LOC, cfg.HID, cfg.KS
    PAIRS = D_LOC // 2
    BLK = 8 if D_LOC % 8 == 0 else D_LOC     # d-planes per conv block
    NB = D_LOC // BLK
    W2 = W // 2
    NT = KS * KS

    xs = nc.dram_tensor("xs", [S, H, W, C], F32, kind="ExternalInput").ap()
    w1 = nc.dram_tensor("w1", [C, HID], F32, kind="ExternalInput").ap()
    b1 = nc.dram_tensor("b1", [1, HID], F32, kind="ExternalInput").ap()
    w2 = nc.dram_tensor("w2", [HID, C], F32, kind="ExternalInput").ap()
    b2 = nc.dram_tensor("b2", [1, C], F32, kind="ExternalInput").ap()
    sband = nc.dram_tensor("sband", [P, NT, H], BF16, kind="ExternalInput").ap()
    out_t = nc.dram_tensor("out", [D_LOC, H, W, C], F32, kind="ExternalOutput").ap()

    groups = [[i, i + 1] for i in range(0, cfg.N_CORES, 2)]

    with tile.TileContext(nc) as tc:
        with (
            tc.tile_pool(name="consts", bufs=1) as consts,
            tc.tile_pool(name="cache", bufs=1) as cachep,
            tc.tile_pool(name="stage", bufs=3) as stagep,
            tc.tile_pool(name="work", bufs=3) as workp,
            tc.tile_pool(name="dram", bufs=1, space="DRAM") as dram,
            tc.tile_pool(name="ps_stats", bufs=1, space="PSUM") as ps_stats,
            tc.tile_pool(name="ps_perm", bufs=2, space="PSUM") as ps_perm,
            tc.tile_pool(name="ps_cv", bufs=2, space="PSUM") as ps_cv,
            tc.tile_pool(name="ps_sm", bufs=1, space="PSUM") as ps_sm,
        ):
            # ---------------- constants ----------------
            ones = consts.tile([P, 1], BF16, tag="ones")
            nc.vector.memset(ones, 1.0)

            # permutation matrices. pooled partition layout is (ci*H + h').
            # QA_e: delta(k=m) for m<H   (avg <- even plane, k<H)
            # QB_e: delta(k=m-H) m>=H    (max <- even plane)
            # QA_o: delta(k=m+H) m<H     (avg <- odd plane, k>=H)
            # QB_o: delta(k=m) m>=H      (max <- odd plane)
            def diag(t, col_lo, col_hi, base):
                nc.gpsimd.affine_select(
                    out=t[:, col_lo:col_hi], in_=t[:, col_lo:col_hi],
                    compare_op=OP.not_equal, fill=1.0, base=base,
                    pattern=[[-1, col_hi - col_lo]], channel_multiplier=1)

            qa_e = consts.tile([P, P], F32, tag="qa_e")
            qb_e = consts.tile([P, P], F32, tag="qb_e")
            qa_o = consts.tile([P, P], F32, tag="qa_o")
            qb_o = consts.tile([P, P], F32, tag="qb_o")
            for t in (qa_e, qb_e, qa_o, qb_o):
                nc.gpsimd.memset(t, 0.0)
            diag(qa_e, 0, H, 0)
            diag(qb_e, H, P, 0)
            diag(qa_o, 0, H, -H)
            diag(qb_o, H, P, -H)
            # bf16 copies of the even-plane selectors for the sa-perm matmuls
            qa_eb = consts.tile([P, P], BF16, tag="qa_eb")
            qb_eb = consts.tile([P, P], BF16, tag="qb_eb")
            nc.gpsimd.tensor_copy(out=qa_eb, in_=qa_e)
            nc.gpsimd.tensor_copy(out=qb_eb, in_=qb_e)

            sband_sb = consts.tile([P, NT, H], BF16, tag="sband")
            nc.sync.dma_start(
                out=sband_sb[:].rearrange("p t h -> p (t h)"),
                in_=sband.rearrange("p t h -> p (t h)"))
            w1_sb = consts.tile([C, HID], F32, tag="w1")
            nc.gpsimd.dma_start(out=w1_sb, in_=w1)
            w2_sb = consts.tile([HID, C], F32, tag="w2")
            nc.gpsimd.dma_start(out=w2_sb, in_=w2)

            def dma_bcast(dst, src_ap, parts):
                a = bass.AP(tensor=src_ap.tensor, offset=src_ap.offset,
                            ap=[[0, parts]] + [list(p) for p in src_ap.ap[1:]])
                nc.gpsimd.dma_start(out=dst, in_=a)

            b1b = consts.tile([2, HID], F32, tag="b1")
            dma_bcast(b1b, b1, 2)
            b2b = consts.tile([2, C], F32, tag="b2")
            dma_bcast(b2b, b2, 2)

            # persistent state
            cache = [cachep.tile([P, W, C], BF16, tag=f"cache{j}", name=f"cache{j}")
                     for j in range(PAIRS)]
            acc_max = cachep.tile([P, W, C], BF16, tag="accmax")
            nc.vector.memset(acc_max, -3.0e38)
            pooled = cachep.tile([P, S, WP], BF16, tag="pooled")
            nc.gpsimd.memset(pooled, 0.0)
            sa_sb = [cachep.tile([H, BLK, W], BF16, tag=f"sa{b}", name=f"sa{b}")
                     for b in range(NB)]

            def load_halfpair(dst_sb, s0, wh):
                """DMA xs[s0:s0+2, :, wh*W2:(wh+1)*W2, :] -> dst_sb [P, W2, C]"""
                nc.sync.dma_start(
                    out=dst_sb[:].rearrange("p w c -> p (w c)"),
                    in_=xs[s0:s0 + 2, :, wh * W2:(wh + 1) * W2, :]
                    .rearrange("d h w c -> (d h) (w c)"))

            # ---------------- pass 1: stream + stats ----------------
            psum_stats = ps_stats.tile([1, 8, C], F32, tag="stats")
            n_wg = W // 8
            mm_i = 0
            n_mm = PAIRS * n_wg
            for j in range(PAIRS):
                s0 = HALO + 2 * j
                for wh in range(2):
                    st = stagep.tile([P, W2, C], F32, tag="stage")
                    load_halfpair(st, s0, wh)
                    nc.gpsimd.tensor_copy(
                        out=cache[j][:, wh * W2:(wh + 1) * W2, :], in_=st)
                for g in range(n_wg):
                    nc.tensor.matmul(
                        out=psum_stats,
                        lhsT=ones[:, :],
                        rhs=cache[j][:, g * 8:(g + 1) * 8, :],
                        start=(mm_i == 0), stop=(mm_i == n_mm - 1))
                    mm_i += 1
                nc.vector.tensor_tensor(
                    out=acc_max, in0=acc_max, in1=cache[j], op=OP.max)

            # finalize stats
            sumc = workp.tile([1, C], F32, tag="sumc")
            nc.vector.tensor_reduce(
                out=sumc, in_=psum_stats[:, :, :].transpose([0, 2, 1]),
                axis=AX.X, op=OP.add)
            maxc_t = workp.tile([P, C], F32, tag="maxct")
            nc.vector.tensor_reduce(
                out=maxc_t, in_=acc_max[:, :, :].transpose([0, 2, 1]),
                axis=AX.X, op=OP.max)
            maxr = workp.tile([P, C], F32, tag="maxr")
            nc.gpsimd.partition_all_reduce(
                out_ap=maxr, in_ap=maxc_t, channels=P,
                reduce_op=bass_isa.ReduceOp.max)

            snd_s = dram.tile([1, C], F32, tag="snd_s")
            rcv_s = dram.tile([1, C], F32, tag="rcv_s")
            snd_m = dram.tile([1, C], F32, tag="snd_m")
            rcv_m = dram.tile([1, C], F32, tag="rcv_m")
            nc.gpsimd.dma_start(out=snd_s, in_=sumc)
            nc.gpsimd.dma_start(out=snd_m, in_=maxr[0:1, :])
            if cfg.use_collectives:
                nc.gpsimd.collective_compute(
                    "AllReduce", OP.add, replica_groups=groups,
                    ins=[snd_s.opt()], outs=[rcv_s.opt()])
                nc.gpsimd.collective_compute(
                    "AllReduce", OP.max, replica_groups=groups,
                    ins=[snd_m.opt()], outs=[rcv_m.opt()])
            else:
                nc.gpsimd.dma_start(out=rcv_s, in_=snd_s)
                nc.gpsimd.dma_start(out=rcv_m, in_=snd_m)

            # ---------------- MLP -> ca ----------------
            if cfg.stop_after == "pass1":
                return nc
            pooled2 = workp.tile([C, 2], F32, tag="pooled2")
            nc.gpsimd.dma_start(out=pooled2[:, 0:1], in_=rcv_s[:, :])
            nc.scalar.mul(out=pooled2[:, 0:1], in_=pooled2[:, 0:1],
                          mul=1.0 / float(cfg.D_TOT * H * W))
            nc.gpsimd.dma_start(out=pooled2[:, 1:2], in_=rcv_m[:, :])

            psum_h = ps_sm.tile([2, HID], F32, tag="small")
            nc.tensor.matmul(out=psum_h, lhsT=pooled2, rhs=w1_sb,
                             start=True, stop=True)
            h_tmp = workp.tile([2, HID], F32, tag="h_tmp")
            nc.vector.tensor_add(out=h_tmp, in0=psum_h, in1=b1b)
            h_sb = workp.tile([2, HID], F32, tag="h")
            nc.scalar.activation(out=h_sb, in_=h_tmp, func=ACT.Relu)
            hT = workp.tile([HID, 2], F32, tag="hT")
            nc.gpsimd.dma_start(out=hT[:, 0:1], in_=h_sb[0:1, :])
            nc.gpsimd.dma_start(out=hT[:, 1:2], in_=h_sb[1:2, :])
            psum_ca = ps_sm.tile([2, C], F32, tag="small")
            nc.tensor.matmul(out=psum_ca, lhsT=hT, rhs=w2_sb,
                             start=True, stop=True)
            ca_tmp = workp.tile([2, C], F32, tag="ca_tmp")
            nc.vector.tensor_add(out=ca_tmp, in0=psum_ca, in1=b2b)
            ca2 = workp.tile([2, C], F32, tag="ca2")
            nc.scalar.activation(out=ca2, in_=ca_tmp, func=ACT.Sigmoid)
            car = workp.tile([2, C], F32, tag="car")
            nc.gpsimd.partition_all_reduce(
                out_ap=car, in_ap=ca2, channels=2,
                reduce_op=bass_isa.ReduceOp.add)
            ca_b = consts.tile([P, C], F32, tag="ca_b")
            nc.gpsimd.partition_broadcast(out_ap=ca_b, in_ap=car[0:1, :])
            ca_bf = consts.tile([P, C], BF16, tag="ca_bf")
            nc.vector.tensor_copy(out=ca_bf, in_=ca_b)

            # ---------------- phase 2a: pooled map ----------------
            if cfg.stop_after == "mlp":
                return nc
            def pool_pair(rsum_a, rsum_m, slot_e, slot_o):
                pe = ps_perm.tile([P, W], F32, tag="perm")
                nc.tensor.matmul(out=pe, lhsT=qa_e,
                                 rhs=rsum_a, start=True, stop=False)
                nc.tensor.matmul(out=pe, lhsT=qb_e,
                                 rhs=rsum_m, start=False, stop=True)
                nc.scalar.copy(out=pooled[:, slot_e, HALO:HALO + W], in_=pe)
                po = ps_perm.tile([P, W], F32, tag="perm")
                nc.tensor.matmul(out=po, lhsT=qa_o,
                                 rhs=rsum_a, start=True, stop=False)
                nc.tensor.matmul(out=po, lhsT=qb_o,
                                 rhs=rsum_m, start=False, stop=True)
                nc.scalar.copy(out=pooled[:, slot_o, HALO:HALO + W], in_=po)

            # own planes (from bf16 cache; writes refined in-place)
            for j in range(PAIRS):
                nc.vector.tensor_mul(
                    out=cache[j], in0=cache[j],
                    in1=_bc(ca_bf[:, :], [P, W, C], 1))
                rsum_a = workp.tile([P, W], F32, tag="rsum_a")
                nc.vector.tensor_reduce(out=rsum_a, in_=cache[j],
                                        axis=AX.X, op=OP.add)
                rsum_m = workp.tile([P, W], F32, tag="rsum_m")
                nc.vector.tensor_reduce(out=rsum_m, in_=cache[j],
                                        axis=AX.X, op=OP.max)
                pool_pair(rsum_a, rsum_m, HALO + 2 * j, HALO + 2 * j + 1)

            # halo pairs: slots (0,1) and (S-2,S-1)
            for s0 in (0, S - 2):
                rsum_a = workp.tile([P, W], F32, tag="rsum_a")
                rsum_m = workp.tile([P, W], F32, tag="rsum_m")
                for wh in range(2):
                    st = stagep.tile([P, W2, C], F32, tag="stage")
                    load_halfpair(st, s0, wh)
                    nc.vector.tensor_mul(out=st, in0=st,
                                         in1=_bc(ca_b[:, :], [P, W2, C], 1))
                    nc.vector.tensor_reduce(
                        out=rsum_a[:, wh * W2:(wh + 1) * W2], in_=st,
                        axis=AX.X, op=OP.add)
                    nc.vector.tensor_reduce(
                        out=rsum_m[:, wh * W2:(wh + 1) * W2], in_=st,
                        axis=AX.X, op=OP.max)
                pool_pair(rsum_a, rsum_m, s0, s0 + 1)

            # halo singles: slots HALO-1 and S-HALO
            for s in (HALO - 1, S - HALO):
                ravg = workp.tile([H, W], F32, tag="ravg_s")
                rmax = workp.tile([H, W], F32, tag="rmax_s")
                for wh in range(2):
                    st = stagep.tile([P, W2, C], F32, tag="stage")
                    sts = st[0:H, :, :]
                    nc.sync.dma_start(
                        out=sts.rearrange("p w c -> p (w c)"),
                        in_=xs[s:s + 1, :, wh * W2:(wh + 1) * W2, :]
                        .rearrange("d h w c -> (d h) (w c)"))
                    nc.vector.tensor_mul(out=sts, in0=sts,
                                         in1=_bc(ca_b[0:H, :], [H, W2, C], 1))
                    nc.vector.tensor_reduce(
                        out=ravg[:, wh * W2:(wh + 1) * W2], in_=sts,
                        axis=AX.X, op=OP.add)
                    nc.vector.tensor_reduce(
                        out=rmax[:, wh * W2:(wh + 1) * W2], in_=sts,
                        axis=AX.X, op=OP.max)
                pss = ps_perm.tile([P, W], F32, tag="perm")
                nc.tensor.matmul(out=pss, lhsT=qa_e[0:H, :],
                                 rhs=ravg, start=True, stop=False)
                nc.tensor.matmul(out=pss, lhsT=qb_e[0:H, :],
                                 rhs=rmax, start=False, stop=True)
                nc.scalar.copy(out=pooled[:, s, HALO:HALO + W], in_=pss)

            # ---------------- conv + sigmoid ----------------
            if cfg.stop_after == "pool":
                return nc
            for blk in range(NB):
                pcv = ps_cv.tile([H, BLK, W], F32, tag="cv")
                k = 0
                for kd in range(KS):
                    for kw in range(KS):
                        nc.tensor.matmul(
                            out=pcv,
                            lhsT=sband_sb[:, kd * KS + kw, :],
                            rhs=pooled[:, blk * BLK + kd: blk * BLK + kd + BLK,
                                       kw:kw + W],
                            start=(k == 0), stop=(k == NT - 1))
                        k += 1
                nc.scalar.activation(out=sa_sb[blk], in_=pcv, func=ACT.Sigmoid)

            # ---------------- apply + writeback ----------------
            if cfg.stop_after == "conv":
                return nc
            for j in range(PAIRS):
                blk, dd = (2 * j) // BLK, (2 * j) % BLK
                psp = ps_perm.tile([P, W], F32, tag="perm")
                nc.tensor.matmul(out=psp, lhsT=qa_eb[0:H, :],
                                 rhs=sa_sb[blk][:, dd, :],
                                 start=True, stop=False)
                nc.tensor.matmul(out=psp, lhsT=qb_eb[0:H, :],
                                 rhs=sa_sb[blk][:, dd + 1, :],
                                 start=False, stop=True)
                sa_bf = workp.tile([P, W], BF16, tag="sa_bf")
                nc.vector.tensor_copy(out=sa_bf, in_=psp)
                for wh in range(2):
                    sto = stagep.tile([P, W2, C], F32, tag="stage")
                    nc.vector.tensor_mul(
                        out=sto, in0=cache[j][:, wh * W2:(wh + 1) * W2, :],
                        in1=_bc(sa_bf[:, wh * W2:(wh + 1) * W2], [P, W2, C], 2))
                    nc.sync.dma_start(
                        out=out_t[2 * j:2 * j + 2, :, wh * W2:(wh + 1) * W2, :]
                        .rearrange("d h w c -> (d h) (w c)"),
                        in_=sto[:].rearrange("p w c -> p (w c)"))
    return nc


def make_sband(conv_w, cfg: Cfg):
    """Host-side band-matrix construction: [P, KS*KS, H] bf16.

    sband[ci*H+h', kd*KS+kw, h] = conv_w[kd, h'-h+halo, kw, ci] (avg rows
    pre-scaled by 1/C because the pooled map stores channel sums)."""
    H, C, KS, HALO = cfg.H, cfg.C, cfg.KS, cfg.HALO
    cw = np.asarray(conv_w, np.float32)[..., 0]        # [KS,KS,KS,2]
    sb = np.zeros((cfg.P, KS * KS, H), np.float32)
    h = np.arange(H)
    for kd in range(KS):
        for kw in range(KS):
            for ci in range(2):
                scale = (1.0 / C) if ci == 0 else 1.0
                for kh in range(KS):
                    hp = h + kh - HALO                  # h' = h + kh - halo
                    m = (hp >= 0) & (hp < H)
                    sb[ci * H + hp[m], kd * KS + kw, h[m]] = cw[kd, kh, kw, ci] * scale
    return sb.astype(ml_dtypes.bfloat16)


def make_core_inputs(x, w1, b1, w2, b2, sband_np, cfg: Cfg):
    """Shard the full inputs into per-core in_maps."""
    H, W, C, D_LOC, HALO, S = cfg.H, cfg.W, cfg.C, cfg.D_LOC, cfg.HALO, cfg.S
    x = np.ascontiguousarray(np.asarray(x, np.float32))
    B, D = x.shape[0], x.shape[1]
    in_maps = []
    for core in range(cfg.N_CORES):
        b, half = core // 2, core % 2
        d0 = half * D_LOC
        xsl = np.zeros((S, H, W, C), np.float32)
        xsl[HALO:HALO + D_LOC] = x[b, d0:d0 + D_LOC]
        if d0 > 0:
            xsl[:HALO] = x[b, d0 - HALO:d0]
        if d0 + D_LOC < D:
            xsl[HALO + D_LOC:] = x[b, d0 + D_LOC:d0 + D_LOC + HALO]
        in_maps.append({
            "xs": xsl,
            "w1": np.asarray(w1, np.float32).reshape(C, cfg.HID),
            "b1": np.asarray(b1, np.float32).reshape(1, cfg.HID),
            "w2": np.asarray(w2, np.float32).reshape(cfg.HID, C),
            "b2": np.asarray(b2, np.float32).reshape(1, C),
            "sband": sband_np,
        })
    return in_maps


_COMPILED = {}


def get_compiled(cfg: Cfg = FULL):
    if cfg not in _COMPILED:
        nc = bacc.Bacc("TRN2", target_bir_lowering=False, debug=False,
                       num_devices=cfg.N_CORES)
        build_cbam(nc, cfg)
        nc.compile()
        _COMPILED[cfg] = nc
    return _COMPILED[cfg]


def kernel(x, w1, b1, w2, b2, conv_w):
    from concourse.bass_utils import run_bass_kernel_spmd

    cfg = FULL
    nc = get_compiled(cfg)
    sband_np = make_sband(conv_w, cfg)
    in_maps = make_core_inputs(x, w1, b1, w2, b2, sband_np, cfg)
    res = run_bass_kernel_spmd(nc, in_maps, list(range(cfg.N_CORES)))
    B, D = 4, 64
    out = np.empty((B, D, cfg.H, cfg.W, cfg.C), np.float32)
    for core in range(cfg.N_CORES):
        b, half = core // 2, core % 2
        d0 = half * cfg.D_LOC
        out[b, d0:d0 + cfg.D_LOC] = res.results[core]["out"]
    return out
